# revision 12
# baseline (speedup 1.0000x reference)
"""Attention-only Llama forward on 8 trn2 NeuronCores.

Sharding: 2 batch groups x 4-core tensor-parallel head groups.
Core c handles batch b = c//4 and heads [4g:4g+4], g = c%4.

Per-core dataflow (all activations transposed: xT [D, T] with D on
partitions as 8 tiles of 128):
  - embedding: device indirect-DMA gather over a host-deduplicated table,
    then PE transposes into xT.
  - q/k projections emit even/odd RoPE components separately via
    host-permuted weight columns, so RoPE is 6 partition-aligned DVE ops
    per tensor covering all 4 heads at once.
  - scores computed transposed (scoresT [k, q]) as K=32 matmul pairs per
    head; causal masking by skipping fully-masked blocks + one triangular
    mask tile on diagonal blocks.
  - softmax denominator via a ones-column appended to v (row 64 of the
    attention-output accumulator), broadcast across partitions with a K=1
    ones matmul.
  - o_proj partials all-reduced in bf16 across each 4-core head group.
  - final RMSNorm in fp32 after PE-transposing back to [T, D].

Matmul inputs are bf16 (1 cycle/row on PE); all accumulation is fp32.
"""

import json
import math
import os
import sys

for _p in ("/opt/trn_rl_repo", "/root/.axon_site/_ro/trn_rl_repo"):
    if os.path.isdir(_p) and _p not in sys.path:
        sys.path.insert(0, _p)

import ml_dtypes
import numpy as np

import concourse.bass as bass
import concourse.tile as tile
from concourse import mybir
from concourse.bass import ds, ts
from concourse.bass_utils import run_bass_kernel_spmd
from concourse.masks import make_identity

L = 4
D = 1024
H = 16
HD = 64
V = 32000
B = 2
T = 1024
THETA = 10000.0
EPS = 1e-5

N_CORES = 8
GROUPS = [[0, 1, 2, 3], [4, 5, 6, 7]]
HPC = 4          # heads per core
C = HPC * HD     # head-col slice width per core = 256

F32 = mybir.dt.float32
BF16 = mybir.dt.bfloat16
I32 = mybir.dt.int32
Exp = mybir.ActivationFunctionType.Exp
Square = mybir.ActivationFunctionType.Square
Sqrt = mybir.ActivationFunctionType.Sqrt


# --- walrus compat: this build supports at most one sync wait per
# instruction; split extras onto EventSemaphore insts on the same engine.
def _split_multi_waits(bir: dict) -> int:
    ctr = 0
    for f in bir.get("functions", []):
        for bb in f.get("blocks", []):
            insts = bb.get("instructions", [])
            if not any(
                len((i.get("sync_info") or {}).get("on_wait") or []) > 1
                for i in insts
            ):
                continue
            out = []
            for inst in insts:
                si = inst.get("sync_info")
                waits = (si or {}).get("on_wait") or []
                if len(waits) > 1:
                    for w in waits[:-1]:
                        ctr += 1
                        out.append({
                            "debug": inst.get("debug"),
                            "engine": inst["engine"],
                            "ins": [],
                            "name": f"WSPLIT-{ctr}",
                            "opcode": "EventSemaphore",
                            "outs": [],
                            "sync_info": {"on_update": [], "on_wait": [w]},
                        })
                    si["on_wait"] = [waits[-1]]
                out.append(inst)
            bb["instructions"] = out
    return ctr


class CompatBass(bass.Bass):
    def to_json_bytes(self) -> bytes:
        raw = super().to_json_bytes()
        bir = json.loads(raw)
        if _split_multi_waits(bir):
            return json.dumps(bir).encode()
        return raw


def _build_program():
    nc = CompatBass(num_devices=N_CORES)

    toks_ext = nc.declare_dram_parameter("toks", [8, 128, 1], I32, isOutput=False)
    emb_ext = nc.declare_dram_parameter("emb", [T, D], F32, isOutput=False)
    wqe_ext = nc.declare_dram_parameter("wqe", [L, 128, 8, 128], BF16, isOutput=False)
    wqo_ext = nc.declare_dram_parameter("wqo", [L, 128, 8, 128], BF16, isOutput=False)
    wke_ext = nc.declare_dram_parameter("wke", [L, 128, 8, 128], BF16, isOutput=False)
    wko_ext = nc.declare_dram_parameter("wko", [L, 128, 8, 128], BF16, isOutput=False)
    wv_ext = nc.declare_dram_parameter("wv", [L, 128, 8, 256], BF16, isOutput=False)
    wo_ext = nc.declare_dram_parameter("wo", [L, 4, 64, D], BF16, isOutput=False)
    cosq_ext = nc.declare_dram_parameter("cosq", [128, T], F32, isOutput=False)
    sinq_ext = nc.declare_dram_parameter("sinq", [128, T], F32, isOutput=False)
    cosk_ext = nc.declare_dram_parameter("cosk", [128, T], F32, isOutput=False)
    sink_ext = nc.declare_dram_parameter("sink", [128, T], F32, isOutput=False)
    mask_ext = nc.declare_dram_parameter("trimask", [128, 128], F32, isOutput=False)
    normw_ext = nc.declare_dram_parameter("normw", [D], F32, isOutput=False)
    out_ext = nc.declare_dram_parameter("out", [T, D], F32, isOutput=True)

    from contextlib import ExitStack

    with tile.TileContext(nc) as tc, ExitStack() as stack:
        ec = stack.enter_context
        cpool = ec(tc.tile_pool(name="consts", bufs=1))
        xpool = ec(tc.tile_pool(name="xT", bufs=2))
        wpool = ec(tc.tile_pool(name="weights", bufs=2))
        qkpool = ec(tc.tile_pool(name="qk", bufs=1))
        vpool = ec(tc.tile_pool(name="v", bufs=1))
        atpool = ec(tc.tile_pool(name="att", bufs=3))
        otpool = ec(tc.tile_pool(name="oT", bufs=1))
        xppool = ec(tc.tile_pool(name="xpart", bufs=2))
        gpool = ec(tc.tile_pool(name="gather", bufs=2))
        npool = ec(tc.tile_pool(name="norm", bufs=1))
        mpool = ec(tc.tile_pool(name="misc", bufs=1))
        dpool = ec(tc.tile_pool(name="dram", bufs=2, space="DRAM"))
        pspool = ec(tc.tile_pool(name="ps_mm", bufs=2, space="PSUM"))
        scpool = ec(tc.tile_pool(name="ps_sc", bufs=3, space="PSUM"))
        oaccpool = ec(tc.tile_pool(name="ps_oacc", bufs=2, space="PSUM"))
        if True:
            identf = cpool.tile([128, 128], F32, tag="identf")
            make_identity(nc, identf)
            identb = cpool.tile([128, 128], BF16, tag="identb")
            make_identity(nc, identb)
            trimask = cpool.tile([128, 128], F32, tag="trimask")
            nc.sync.dma_start(out=trimask[:], in_=mask_ext[:])
            cosq = cpool.tile([128, T], F32, tag="cosq")
            nc.sync.dma_start(out=cosq[:], in_=cosq_ext[:])
            sinq = cpool.tile([128, T], F32, tag="sinq")
            nc.sync.dma_start(out=sinq[:], in_=sinq_ext[:])
            cosk = cpool.tile([128, T], F32, tag="cosk")
            nc.sync.dma_start(out=cosk[:], in_=cosk_ext[:])
            sink = cpool.tile([128, T], F32, tag="sink")
            nc.sync.dma_start(out=sink[:], in_=sink_ext[:])
            normw_b = cpool.tile([128, D], F32, tag="normw")
            _nw = normw_ext[:]
            nc.sync.dma_start(
                out=normw_b[:],
                in_=bass.AP(tensor=_nw.tensor, offset=_nw.offset,
                            ap=[[0, 128], list(_nw.ap[0])]),
            )
            ones64 = cpool.tile([1, 64], F32, tag="ones64")
            nc.vector.memset(ones64[:], 1.0)
            epst = cpool.tile([128, 1], F32, tag="epst")
            nc.vector.memset(epst[:], EPS)

            # ---- embedding gather + transpose into xT [128, 8, 1024] bf16
            xT = xpool.tile([128, 8, T], BF16, tag="xT")
            for j in range(8):
                idx = gpool.tile([128, 1], I32, tag="idx")
                nc.sync.dma_start(out=idx[:], in_=toks_ext[j])
                xg = gpool.tile([128, D], F32, tag="xg")
                nc.gpsimd.indirect_dma_start(
                    out=xg[:], out_offset=None, in_=emb_ext[:],
                    in_offset=bass.IndirectOffsetOnAxis(ap=idx[:, :1], axis=0),
                )
                for k in range(8):
                    tp = scpool.tile([128, 128], F32, tag="sc")
                    nc.tensor.transpose(tp[:], xg[:, ts(k, 128)], identf[:])
                    nc.vector.tensor_copy(out=xT[:, k, ts(j, 128)], in_=tp[:])

            for l in range(L):
                # ---- weight loads
                wqe = wpool.tile([128, 8, 128], BF16, tag="wqe")
                nc.sync.dma_start(out=wqe[:], in_=wqe_ext[l])
                wqo = wpool.tile([128, 8, 128], BF16, tag="wqo")
                nc.sync.dma_start(out=wqo[:], in_=wqo_ext[l])
                wke = wpool.tile([128, 8, 128], BF16, tag="wke")
                nc.sync.dma_start(out=wke[:], in_=wke_ext[l])
                wko = wpool.tile([128, 8, 128], BF16, tag="wko")
                nc.sync.dma_start(out=wko[:], in_=wko_ext[l])
                wv = wpool.tile([128, 8, 256], BF16, tag="wv")
                nc.sync.dma_start(out=wv[:], in_=wv_ext[l])
                wo = []
                for h in range(4):
                    woh = wpool.tile([64, D], BF16, tag=f"wo{h}")
                    nc.sync.dma_start(out=woh[:], in_=wo_ext[l, h])
                    wo.append(woh)

                # ---- q/k projections (even/odd components, all 4 heads)
                comps = {}
                for name, wt in (("qe", wqe), ("qo", wqo), ("ke", wke), ("ko", wko)):
                    dst = qkpool.tile([128, T], F32, tag=name)
                    for c2 in range(2):
                        ps = pspool.tile([128, 512], F32, tag="mm")
                        for k in range(8):
                            nc.tensor.matmul(
                                ps[:], lhsT=wt[:, k, :],
                                rhs=xT[:, k, ds(512 * c2, 512)],
                                start=(k == 0), stop=(k == 7),
                            )
                        nc.vector.tensor_copy(out=dst[:, ds(512 * c2, 512)], in_=ps[:])
                    comps[name] = dst

                # ---- RoPE (6 DVE ops per tensor; rotated outputs in bf16)
                rot = {}
                for pre, cs, sn in (("q", cosq, sinq), ("k", cosk, sink)):
                    e_in, o_in = comps[pre + "e"], comps[pre + "o"]
                    re = qkpool.tile([128, T], BF16, tag=f"r{pre}e")
                    ro = qkpool.tile([128, T], BF16, tag=f"r{pre}o")
                    ta = qkpool.tile([128, T], F32, tag="tmpa")
                    tb = qkpool.tile([128, T], F32, tag="tmpb")
                    nc.vector.tensor_mul(ta[:], e_in[:], cs[:])
                    nc.vector.tensor_mul(tb[:], o_in[:], sn[:])
                    nc.vector.tensor_tensor(
                        out=re[:], in0=ta[:], in1=tb[:], op=mybir.AluOpType.subtract)
                    tc2 = qkpool.tile([128, T], F32, tag="tmpa")
                    td = qkpool.tile([128, T], F32, tag="tmpb")
                    nc.vector.tensor_mul(tc2[:], e_in[:], sn[:])
                    nc.vector.tensor_mul(td[:], o_in[:], cs[:])
                    nc.vector.tensor_tensor(
                        out=ro[:], in0=tc2[:], in1=td[:], op=mybir.AluOpType.add)
                    rot[pre + "e"], rot[pre + "o"] = re, ro

                # head 3 lives at partition base 96 (not addressable by the
                # PE); copy its 32 rows to base-0 tiles.
                r3 = {}
                for name in ("qe", "qo", "ke", "ko"):
                    t3 = qkpool.tile([32, T], BF16, tag=f"r3{name}")
                    nc.vector.tensor_copy(out=t3[:], in_=rot[name][96:128, :])
                    r3[name] = t3

                # ---- v projection, [t, c] layout, +ones column per head
                vt = []
                for j in range(8):
                    ps = pspool.tile([128, 256], F32, tag="mm")
                    for k in range(8):
                        nc.tensor.matmul(
                            ps[:], lhsT=xT[:, k, ts(j, 128)], rhs=wv[:, k, :],
                            start=(k == 0), stop=(k == 7),
                        )
                    vj = vpool.tile([128, 4, 65], BF16, tag=f"v{j}")
                    nc.vector.memset(vj[:, :, 64:65], 1.0)
                    nc.vector.tensor_copy(
                        out=vj[:, :, 0:64],
                        in_=ps[:].rearrange("p (h d) -> p h d", h=4))
                    vt.append(vj)

                # ---- attention per head
                oT = []
                for h in range(4):
                    if h < 3:
                        sl = slice(32 * h, 32 * h + 32)
                        lqe, lqo = rot["qe"][sl, :], rot["qo"][sl, :]
                        lke, lko = rot["ke"][sl, :], rot["ko"][sl, :]
                    else:
                        lqe, lqo = r3["qe"][:], r3["qo"][:]
                        lke, lko = r3["ke"][:], r3["ko"][:]
                    oTh = otpool.tile([64, T], BF16, tag=f"oT{h}")
                    for qc in range(2):
                        oacc = oaccpool.tile([65, 512], F32, tag="oacc")
                        jbmax = 3 if qc == 0 else 7
                        for jb in range(jbmax + 1):
                            q_lo = max(512 * qc, 128 * jb)
                            n = 512 * (qc + 1) - q_lo
                            sc = scpool.tile([128, 512], F32, tag="sc")
                            nc.tensor.matmul(
                                sc[:, :n], lhsT=lke[:, ts(jb, 128)],
                                rhs=lqe[:, ds(q_lo, n)], start=True, stop=False)
                            nc.tensor.matmul(
                                sc[:, :n], lhsT=lko[:, ts(jb, 128)],
                                rhs=lqo[:, ds(q_lo, n)], start=False, stop=True)
                            if q_lo == 128 * jb:
                                nc.vector.tensor_add(
                                    out=sc[:, 0:128], in0=sc[:, 0:128],
                                    in1=trimask[:])
                            at = atpool.tile([128, 512], BF16, tag="att")
                            nc.scalar.activation(out=at[:, :n], in_=sc[:, :n], func=Exp)
                            nc.tensor.matmul(
                                oacc[:, ds(q_lo - 512 * qc, n)],
                                lhsT=vt[jb][:, h, :], rhs=at[:, :n],
                                start=(jb == 0), stop=(jb == jbmax))
                        recip = mpool.tile([1, 512], F32, tag="recip")
                        nc.vector.reciprocal(out=recip[:], in_=oacc[64:65, :])
                        bc_ps = scpool.tile([64, 512], F32, tag="sc")
                        nc.tensor.matmul(
                            bc_ps[:], lhsT=ones64[:], rhs=recip[:],
                            start=True, stop=True)
                        bc_sb = mpool.tile([64, 512], F32, tag="bcsb")
                        nc.scalar.copy(out=bc_sb[:], in_=bc_ps[:])
                        nc.vector.tensor_mul(
                            oTh[:, ds(512 * qc, 512)], oacc[0:64, :], bc_sb[:])
                    oT.append(oTh)

                # ---- o_proj partials
                xpart = xppool.tile([128, 8, T], BF16, tag="xp")
                for e in range(8):
                    for c2 in range(2):
                        ps = pspool.tile([128, 512], F32, tag="mm")
                        for h in range(4):
                            nc.tensor.matmul(
                                ps[:], lhsT=wo[h][:, ts(e, 128)],
                                rhs=oT[h][:, ds(512 * c2, 512)],
                                start=(h == 0), stop=(h == 3))
                        nc.vector.tensor_copy(
                            out=xpart[:, e, ds(512 * c2, 512)], in_=ps[:])

                # ---- all-reduce partials across the 4-core head group
                arin = dpool.tile([128, 8, T], BF16, tag="arin")
                arout = dpool.tile([128, 8, T], BF16, tag="arout")
                nc.sync.dma_start(out=arin[:], in_=xpart[:])
                nc.gpsimd.collective_compute(
                    "AllReduce", mybir.AluOpType.add,
                    ins=[arin[:]], outs=[arout[:]], replica_groups=GROUPS)
                xT = xpool.tile([128, 8, T], BF16, tag="xT")
                nc.sync.dma_start(out=xT[:], in_=arout[:])

            # ---- final RMSNorm (fp32) back in [T, D] layout
            for j in range(8):
                xrow = npool.tile([128, D], F32, tag="xrow")
                for k in range(8):
                    tp = scpool.tile([128, 128], BF16, tag="sc")
                    nc.tensor.transpose(tp[:], xT[:, k, ts(j, 128)], identb[:])
                    nc.vector.tensor_copy(out=xrow[:, ts(k, 128)], in_=tp[:])
                onorm = npool.tile([128, D], F32, tag="onorm")
                ssq = npool.tile([128, 1], F32, tag="ssq")
                nc.scalar.activation(out=onorm[:], in_=xrow[:], func=Square,
                                     accum_out=ssq[:])
                std = npool.tile([128, 1], F32, tag="std")
                nc.scalar.activation(out=std[:], in_=ssq[:], func=Sqrt,
                                     scale=1.0 / D, bias=epst[:, :1])
                rstd = npool.tile([128, 1], F32, tag="rstd")
                nc.vector.reciprocal(out=rstd[:], in_=std[:])
                nc.vector.tensor_scalar_mul(out=xrow[:], in0=xrow[:],
                                            scalar1=rstd[:, :1])
                nc.vector.tensor_mul(onorm[:], xrow[:], normw_b[:])
                nc.sync.dma_start(out=out_ext[ts(j, 128), :], in_=onorm[:])

    return nc


def _prep_inputs(toks, embed, Wq, Wk, Wv, Wo, norm_w):
    """Build the 8 per-core input maps from the full model inputs."""
    toks = np.asarray(toks)
    embed = np.asarray(embed, dtype=np.float32)
    Wq = np.asarray(Wq, dtype=np.float32)
    Wk = np.asarray(Wk, dtype=np.float32)
    Wv = np.asarray(Wv, dtype=np.float32)
    Wo = np.asarray(Wo, dtype=np.float32)
    norm_w = np.asarray(norm_w, dtype=np.float32)

    inv = 1.0 / (THETA ** (np.arange(0, HD, 2, dtype=np.float32) / HD))  # [32]
    ang = inv[:, None] * np.arange(T, dtype=np.float32)[None, :]         # [32, T]
    cos = np.cos(ang)
    sin = np.sin(ang)
    scale = 1.0 / math.sqrt(HD)
    cosq = np.tile(cos * scale, (4, 1)).astype(np.float32)
    sinq = np.tile(sin * scale, (4, 1)).astype(np.float32)
    cosk = np.tile(cos, (4, 1)).astype(np.float32)
    sink = np.tile(sin, (4, 1)).astype(np.float32)

    jj = np.arange(128)
    trimask = np.where(jj[:, None] <= jj[None, :], 0.0, -1e9).astype(np.float32)

    in_maps = []
    batch_tables = []
    for b in range(B):
        uniq, invmap = np.unique(np.asarray(toks[b], dtype=np.int64),
                                 return_inverse=True)
        table = np.zeros((T, D), dtype=np.float32)
        table[: len(uniq)] = embed[uniq]
        batch_tables.append((table, invmap.astype(np.int32)))

    for c in range(N_CORES):
        b, g = c // 4, c % 4
        table, invmap = batch_tables[b]
        heads = [4 * g + h for h in range(4)]
        ecols = np.concatenate([64 * ah + np.arange(0, 64, 2) for ah in heads])
        ocols = np.concatenate([64 * ah + np.arange(1, 64, 2) for ah in heads])
        vcols = np.arange(256 * g, 256 * g + 256)

        def tile_w(w):  # [L, D, 128 or 256] -> [L, 128, 8, n]
            n = w.shape[-1]
            return np.ascontiguousarray(
                w.reshape(L, 8, 128, n).transpose(0, 2, 1, 3)
            ).astype(ml_dtypes.bfloat16)

        in_maps.append({
            "toks": invmap.reshape(8, 128, 1),
            "emb": table,
            "wqe": tile_w(Wq[:, :, ecols]),
            "wqo": tile_w(Wq[:, :, ocols]),
            "wke": tile_w(Wk[:, :, ecols]),
            "wko": tile_w(Wk[:, :, ocols]),
            "wv": tile_w(Wv[:, :, vcols]),
            "wo": np.ascontiguousarray(
                Wo[:, vcols, :].reshape(L, 4, 64, D)).astype(ml_dtypes.bfloat16),
            "cosq": cosq, "sinq": sinq, "cosk": cosk, "sink": sink,
            "trimask": trimask,
            "normw": norm_w,
        })
    return in_maps


class _Runner:
    """Compile the SPMD program once; re-executable on the 8 cores.

    Mirrors concourse.bass2jax.run_bass_via_pjrt but caches the jitted
    shard_map so repeated calls don't re-trace or re-compile.
    """

    def __init__(self):
        import jax
        from jax.sharding import Mesh, PartitionSpec

        try:
            from jax.experimental.shard_map import shard_map
        except ImportError:
            from jax.shard_map import shard_map

        from concourse import bass2jax

        bass2jax.install_neuronx_cc_hook()
        nc = _build_program()
        self._jax = jax

        partition_name = (
            nc.partition_id_tensor.name if nc.partition_id_tensor else None
        )
        in_names, out_names, out_avals, zero_outs = [], [], [], []
        for alloc in nc.m.functions[0].allocations:
            if not isinstance(alloc, mybir.MemoryLocationSet):
                continue
            name = alloc.memorylocations[0].name
            if alloc.kind == "ExternalInput":
                if name != partition_name:
                    in_names.append(name)
            elif alloc.kind == "ExternalOutput":
                out_names.append(name)
                shape = tuple(alloc.tensor_shape)
                dtype = mybir.dt.np(alloc.dtype)
                out_avals.append(jax.core.ShapedArray(shape, dtype))
                zero_outs.append(np.zeros(shape, dtype))
        self.in_names = list(in_names)
        self.out_names = out_names
        n_params = len(in_names)
        all_in_names = in_names + out_names
        if partition_name is not None:
            all_in_names = all_in_names + [partition_name]

        def _body(*args):
            operands = list(args)
            if partition_name is not None:
                operands.append(bass2jax.partition_id_tensor())
            outs = bass2jax._bass_exec_p.bind(
                *operands,
                out_avals=tuple(out_avals),
                in_names=tuple(all_in_names),
                out_names=tuple(out_names),
                lowering_input_output_aliases=(),
                sim_require_finite=True,
                sim_require_nnan=True,
                nc=nc,
            )
            return tuple(outs)

        devices = jax.devices()[:N_CORES]
        mesh = Mesh(np.asarray(devices), ("core",))
        in_specs = (PartitionSpec("core"),) * (n_params + len(out_names))
        out_specs = (PartitionSpec("core"),) * len(out_names)
        self._fn = jax.jit(
            shard_map(_body, mesh=mesh, in_specs=in_specs,
                      out_specs=out_specs, check_rep=False),
            keep_unused=True,
        )
        self._zero_outs = zero_outs
        self._out_avals = out_avals

    def place(self, in_maps):
        cat = [
            np.concatenate([np.asarray(in_maps[c][n]) for c in range(N_CORES)],
                           axis=0)
            for n in self.in_names
        ]
        cat += [
            np.zeros((N_CORES * z.shape[0], *z.shape[1:]), z.dtype)
            for z in self._zero_outs
        ]
        return cat

    def execute(self, placed):
        return self._fn(*placed)

    def run(self, in_maps):
        out_arrs = self.execute(self.place(in_maps))
        return [
            {
                n: np.asarray(out_arrs[i]).reshape(
                    N_CORES, *self._out_avals[i].shape)[c]
                for i, n in enumerate(self.out_names)
            }
            for c in range(N_CORES)
        ]


_CACHE = {}


def get_runner():
    if "runner" not in _CACHE:
        _CACHE["runner"] = _Runner()
    return _CACHE["runner"]


def kernel(toks, embed, Wq, Wk, Wv, Wo, norm_w):
    in_maps = _prep_inputs(toks, embed, Wq, Wk, Wv, Wo, norm_w)
    results = get_runner().run(in_maps)
    out = np.stack([results[0]["out"], results[4]["out"]])
    return out.astype(np.float32)


# revision 13
# speedup vs baseline: 339.6967x; 339.6967x over previous
"""Attention-only Llama forward on 8 trn2 NeuronCores.

Sharding: 2 batch groups x 4-core tensor-parallel head groups.
Core c handles batch b = c//4 and heads [4g:4g+4], g = c%4.

Per-core dataflow (all activations transposed: xT [D, T] with D on
partitions as 8 tiles of 128):
  - embedding: device indirect-DMA gather over a host-deduplicated table,
    then PE transposes into xT.
  - q/k projections emit even/odd RoPE components separately via
    host-permuted weight columns, so RoPE is 6 partition-aligned DVE ops
    per tensor covering all 4 heads at once.
  - scores computed transposed (scoresT [k, q]) as K=32 matmul pairs per
    head; causal masking by skipping fully-masked blocks + one triangular
    mask tile on diagonal blocks.
  - softmax denominator via a ones-column appended to v (row 64 of the
    attention-output accumulator), broadcast across partitions with a K=1
    ones matmul.
  - o_proj partials all-reduced in bf16 across each 4-core head group.
  - final RMSNorm in fp32 after PE-transposing back to [T, D].

Matmul inputs are bf16 (1 cycle/row on PE); all accumulation is fp32.
"""

import json
import math
import os
import sys

for _p in ("/opt/trn_rl_repo", "/root/.axon_site/_ro/trn_rl_repo"):
    if os.path.isdir(_p) and _p not in sys.path:
        sys.path.insert(0, _p)

import ml_dtypes
import numpy as np

import concourse.bass as bass
import concourse.tile as tile
from concourse import mybir
from concourse.bass import ds, ts
from concourse.bass_utils import run_bass_kernel_spmd
from concourse.masks import make_identity

L = 4
D = 1024
H = 16
HD = 64
V = 32000
B = 2
T = 1024
THETA = 10000.0
EPS = 1e-5

N_CORES = 8
GROUPS = [[0, 1, 2, 3], [4, 5, 6, 7]]
HPC = 4          # heads per core
C = HPC * HD     # head-col slice width per core = 256

F32 = mybir.dt.float32
BF16 = mybir.dt.bfloat16
I32 = mybir.dt.int32
Exp = mybir.ActivationFunctionType.Exp
Square = mybir.ActivationFunctionType.Square
Sqrt = mybir.ActivationFunctionType.Sqrt


# --- walrus compat: this build supports at most one sync wait per
# instruction; split extras onto EventSemaphore insts on the same engine.
def _split_multi_waits(bir: dict) -> int:
    ctr = 0
    for f in bir.get("functions", []):
        for bb in f.get("blocks", []):
            insts = bb.get("instructions", [])
            if not any(
                len((i.get("sync_info") or {}).get("on_wait") or []) > 1
                for i in insts
            ):
                continue
            out = []
            for inst in insts:
                si = inst.get("sync_info")
                waits = (si or {}).get("on_wait") or []
                if len(waits) > 1:
                    for w in waits[:-1]:
                        ctr += 1
                        out.append({
                            "debug": inst.get("debug"),
                            "engine": inst["engine"],
                            "ins": [],
                            "name": f"WSPLIT-{ctr}",
                            "opcode": "EventSemaphore",
                            "outs": [],
                            "sync_info": {"on_update": [], "on_wait": [w]},
                        })
                    si["on_wait"] = [waits[-1]]
                out.append(inst)
            bb["instructions"] = out
    return ctr


class CompatBass(bass.Bass):
    def to_json_bytes(self) -> bytes:
        raw = super().to_json_bytes()
        bir = json.loads(raw)
        if _split_multi_waits(bir):
            return json.dumps(bir).encode()
        return raw


def _build_program():
    nc = CompatBass(num_devices=N_CORES)

    toks_ext = nc.declare_dram_parameter("toks", [8, 128, 1], I32, isOutput=False)
    emb_ext = nc.declare_dram_parameter("emb", [T, D], F32, isOutput=False)
    wqe_ext = nc.declare_dram_parameter("wqe", [L, 128, 8, 128], BF16, isOutput=False)
    wqo_ext = nc.declare_dram_parameter("wqo", [L, 128, 8, 128], BF16, isOutput=False)
    wke_ext = nc.declare_dram_parameter("wke", [L, 128, 8, 128], BF16, isOutput=False)
    wko_ext = nc.declare_dram_parameter("wko", [L, 128, 8, 128], BF16, isOutput=False)
    wv_ext = nc.declare_dram_parameter("wv", [L, 128, 8, 256], BF16, isOutput=False)
    wo_ext = nc.declare_dram_parameter("wo", [L, 4, 64, D], BF16, isOutput=False)
    cosq_ext = nc.declare_dram_parameter("cosq", [128, T], F32, isOutput=False)
    sinq_ext = nc.declare_dram_parameter("sinq", [128, T], F32, isOutput=False)
    cosk_ext = nc.declare_dram_parameter("cosk", [128, T], F32, isOutput=False)
    sink_ext = nc.declare_dram_parameter("sink", [128, T], F32, isOutput=False)
    mask_ext = nc.declare_dram_parameter("trimask", [128, 128], F32, isOutput=False)
    normw_ext = nc.declare_dram_parameter("normw", [D], F32, isOutput=False)
    out_ext = nc.declare_dram_parameter("out", [T, D], F32, isOutput=True)

    from contextlib import ExitStack

    with tile.TileContext(nc) as tc, ExitStack() as stack:
        ec = stack.enter_context
        cpool = ec(tc.tile_pool(name="consts", bufs=1))
        xpool = ec(tc.tile_pool(name="xT", bufs=2))
        wpool = ec(tc.tile_pool(name="weights", bufs=2))
        qkpool = ec(tc.tile_pool(name="qk", bufs=1))
        vpool = ec(tc.tile_pool(name="v", bufs=1))
        atpool = ec(tc.tile_pool(name="att", bufs=3))
        otpool = ec(tc.tile_pool(name="oT", bufs=1))
        xppool = ec(tc.tile_pool(name="xpart", bufs=2))
        gpool = ec(tc.tile_pool(name="gather", bufs=2))
        npool = ec(tc.tile_pool(name="norm", bufs=1))
        mpool = ec(tc.tile_pool(name="misc", bufs=1))
        dpool = ec(tc.tile_pool(name="dram", bufs=2, space="DRAM"))
        pspool = ec(tc.tile_pool(name="ps_mm", bufs=2, space="PSUM"))
        scpool = ec(tc.tile_pool(name="ps_sc", bufs=3, space="PSUM"))
        oaccpool = ec(tc.tile_pool(name="ps_oacc", bufs=2, space="PSUM"))
        if True:
            identf = cpool.tile([128, 128], F32, tag="identf")
            make_identity(nc, identf)
            identb = cpool.tile([128, 128], BF16, tag="identb")
            make_identity(nc, identb)
            trimask = cpool.tile([128, 128], F32, tag="trimask")
            nc.sync.dma_start(out=trimask[:], in_=mask_ext[:])
            cosq = cpool.tile([128, T], F32, tag="cosq")
            nc.sync.dma_start(out=cosq[:], in_=cosq_ext[:])
            sinq = cpool.tile([128, T], F32, tag="sinq")
            nc.sync.dma_start(out=sinq[:], in_=sinq_ext[:])
            cosk = cpool.tile([128, T], F32, tag="cosk")
            nc.sync.dma_start(out=cosk[:], in_=cosk_ext[:])
            sink = cpool.tile([128, T], F32, tag="sink")
            nc.sync.dma_start(out=sink[:], in_=sink_ext[:])
            normw_b = cpool.tile([128, D], F32, tag="normw")
            _nw = normw_ext[:]
            nc.sync.dma_start(
                out=normw_b[:],
                in_=bass.AP(tensor=_nw.tensor, offset=_nw.offset,
                            ap=[[0, 128], list(_nw.ap[0])]),
            )
            ones64 = cpool.tile([1, 64], F32, tag="ones64")
            nc.vector.memset(ones64[:], 1.0)
            epst = cpool.tile([128, 1], F32, tag="epst")
            nc.vector.memset(epst[:], EPS)

            # ---- embedding gather + transpose into xT [128, 8, 1024] bf16
            xT = xpool.tile([128, 8, T], BF16, tag="xT")
            for j in range(8):
                idx = gpool.tile([128, 1], I32, tag="idx")
                nc.sync.dma_start(out=idx[:], in_=toks_ext[j])
                xg = gpool.tile([128, D], F32, tag="xg")
                nc.gpsimd.indirect_dma_start(
                    out=xg[:], out_offset=None, in_=emb_ext[:],
                    in_offset=bass.IndirectOffsetOnAxis(ap=idx[:, :1], axis=0),
                )
                for k in range(8):
                    tp = scpool.tile([128, 128], F32, tag="sc")
                    nc.tensor.transpose(tp[:], xg[:, ts(k, 128)], identf[:])
                    nc.vector.tensor_copy(out=xT[:, k, ts(j, 128)], in_=tp[:])

            for l in range(L):
                # ---- weight loads
                wqe = wpool.tile([128, 8, 128], BF16, tag="wqe")
                nc.sync.dma_start(out=wqe[:], in_=wqe_ext[l])
                wqo = wpool.tile([128, 8, 128], BF16, tag="wqo")
                nc.sync.dma_start(out=wqo[:], in_=wqo_ext[l])
                wke = wpool.tile([128, 8, 128], BF16, tag="wke")
                nc.sync.dma_start(out=wke[:], in_=wke_ext[l])
                wko = wpool.tile([128, 8, 128], BF16, tag="wko")
                nc.sync.dma_start(out=wko[:], in_=wko_ext[l])
                wv = wpool.tile([128, 8, 256], BF16, tag="wv")
                nc.sync.dma_start(out=wv[:], in_=wv_ext[l])
                wo = []
                for h in range(4):
                    woh = wpool.tile([64, D], BF16, tag=f"wo{h}")
                    nc.sync.dma_start(out=woh[:], in_=wo_ext[l, h])
                    wo.append(woh)

                # ---- q/k projections (even/odd components, all 4 heads)
                comps = {}
                for name, wt in (("qe", wqe), ("qo", wqo), ("ke", wke), ("ko", wko)):
                    dst = qkpool.tile([128, T], F32, tag=name)
                    for c2 in range(2):
                        ps = pspool.tile([128, 512], F32, tag="mm")
                        for k in range(8):
                            nc.tensor.matmul(
                                ps[:], lhsT=wt[:, k, :],
                                rhs=xT[:, k, ds(512 * c2, 512)],
                                start=(k == 0), stop=(k == 7),
                            )
                        nc.vector.tensor_copy(out=dst[:, ds(512 * c2, 512)], in_=ps[:])
                    comps[name] = dst

                # ---- RoPE (6 DVE ops per tensor; rotated outputs in bf16)
                rot = {}
                for pre, cs, sn in (("q", cosq, sinq), ("k", cosk, sink)):
                    e_in, o_in = comps[pre + "e"], comps[pre + "o"]
                    re = qkpool.tile([128, T], BF16, tag=f"r{pre}e")
                    ro = qkpool.tile([128, T], BF16, tag=f"r{pre}o")
                    ta = qkpool.tile([128, T], F32, tag="tmpa")
                    tb = qkpool.tile([128, T], F32, tag="tmpb")
                    nc.vector.tensor_mul(ta[:], e_in[:], cs[:])
                    nc.vector.tensor_mul(tb[:], o_in[:], sn[:])
                    nc.vector.tensor_tensor(
                        out=re[:], in0=ta[:], in1=tb[:], op=mybir.AluOpType.subtract)
                    tc2 = qkpool.tile([128, T], F32, tag="tmpa")
                    td = qkpool.tile([128, T], F32, tag="tmpb")
                    nc.vector.tensor_mul(tc2[:], e_in[:], sn[:])
                    nc.vector.tensor_mul(td[:], o_in[:], cs[:])
                    nc.vector.tensor_tensor(
                        out=ro[:], in0=tc2[:], in1=td[:], op=mybir.AluOpType.add)
                    rot[pre + "e"], rot[pre + "o"] = re, ro

                # head 3 lives at partition base 96 (not addressable by the
                # PE); copy its 32 rows to base-0 tiles.
                r3 = {}
                for name in ("qe", "qo", "ke", "ko"):
                    t3 = qkpool.tile([32, T], BF16, tag=f"r3{name}")
                    nc.vector.tensor_copy(out=t3[:], in_=rot[name][96:128, :])
                    r3[name] = t3

                # ---- v projection, [t, c] layout, +ones column per head
                vt = []
                for j in range(8):
                    ps = pspool.tile([128, 256], F32, tag="mm")
                    for k in range(8):
                        nc.tensor.matmul(
                            ps[:], lhsT=xT[:, k, ts(j, 128)], rhs=wv[:, k, :],
                            start=(k == 0), stop=(k == 7),
                        )
                    vj = vpool.tile([128, 4, 65], BF16, tag=f"v{j}")
                    nc.vector.memset(vj[:, :, 64:65], 1.0)
                    nc.vector.tensor_copy(
                        out=vj[:, :, 0:64],
                        in_=ps[:].rearrange("p (h d) -> p h d", h=4))
                    vt.append(vj)

                # ---- attention per head
                oT = []
                for h in range(4):
                    if h < 3:
                        sl = slice(32 * h, 32 * h + 32)
                        lqe, lqo = rot["qe"][sl, :], rot["qo"][sl, :]
                        lke, lko = rot["ke"][sl, :], rot["ko"][sl, :]
                    else:
                        lqe, lqo = r3["qe"][:], r3["qo"][:]
                        lke, lko = r3["ke"][:], r3["ko"][:]
                    oTh = otpool.tile([64, T], BF16, tag=f"oT{h}")
                    for qc in range(2):
                        oacc = oaccpool.tile([65, 512], F32, tag="oacc")
                        jbmax = 3 if qc == 0 else 7
                        for jb in range(jbmax + 1):
                            q_lo = max(512 * qc, 128 * jb)
                            n = 512 * (qc + 1) - q_lo
                            sc = scpool.tile([128, 512], F32, tag="sc")
                            nc.tensor.matmul(
                                sc[:, :n], lhsT=lke[:, ts(jb, 128)],
                                rhs=lqe[:, ds(q_lo, n)], start=True, stop=False)
                            nc.tensor.matmul(
                                sc[:, :n], lhsT=lko[:, ts(jb, 128)],
                                rhs=lqo[:, ds(q_lo, n)], start=False, stop=True)
                            if q_lo == 128 * jb:
                                nc.vector.tensor_add(
                                    out=sc[:, 0:128], in0=sc[:, 0:128],
                                    in1=trimask[:])
                            at = atpool.tile([128, 512], BF16, tag="att")
                            nc.scalar.activation(out=at[:, :n], in_=sc[:, :n], func=Exp)
                            nc.tensor.matmul(
                                oacc[:, ds(q_lo - 512 * qc, n)],
                                lhsT=vt[jb][:, h, :], rhs=at[:, :n],
                                start=(jb == 0), stop=(jb == jbmax))
                        recip = mpool.tile([1, 512], F32, tag="recip")
                        nc.vector.reciprocal(out=recip[:], in_=oacc[64:65, :])
                        bc_ps = scpool.tile([64, 512], F32, tag="sc")
                        nc.tensor.matmul(
                            bc_ps[:], lhsT=ones64[:], rhs=recip[:],
                            start=True, stop=True)
                        bc_sb = mpool.tile([64, 512], F32, tag="bcsb")
                        nc.scalar.copy(out=bc_sb[:], in_=bc_ps[:])
                        nc.vector.tensor_mul(
                            oTh[:, ds(512 * qc, 512)], oacc[0:64, :], bc_sb[:])
                    oT.append(oTh)

                # ---- o_proj partials
                xpart = xppool.tile([128, 8, T], BF16, tag="xp")
                for e in range(8):
                    for c2 in range(2):
                        ps = pspool.tile([128, 512], F32, tag="mm")
                        for h in range(4):
                            nc.tensor.matmul(
                                ps[:], lhsT=wo[h][:, ts(e, 128)],
                                rhs=oT[h][:, ds(512 * c2, 512)],
                                start=(h == 0), stop=(h == 3))
                        nc.vector.tensor_copy(
                            out=xpart[:, e, ds(512 * c2, 512)], in_=ps[:])

                # ---- all-reduce partials across the 4-core head group
                arin = dpool.tile([128, 8, T], BF16, tag="arin")
                arout = dpool.tile([128, 8, T], BF16, tag="arout")
                nc.sync.dma_start(out=arin[:], in_=xpart[:])
                nc.gpsimd.collective_compute(
                    "AllReduce", mybir.AluOpType.add,
                    ins=[arin[:]], outs=[arout[:]], replica_groups=GROUPS)
                xT = xpool.tile([128, 8, T], BF16, tag="xT")
                nc.sync.dma_start(out=xT[:], in_=arout[:])

            # ---- final RMSNorm (fp32) back in [T, D] layout
            for j in range(8):
                xrow = npool.tile([128, D], F32, tag="xrow")
                for k in range(8):
                    tp = scpool.tile([128, 128], BF16, tag="sc")
                    nc.tensor.transpose(tp[:], xT[:, k, ts(j, 128)], identb[:])
                    nc.vector.tensor_copy(out=xrow[:, ts(k, 128)], in_=tp[:])
                onorm = npool.tile([128, D], F32, tag="onorm")
                ssq = npool.tile([128, 1], F32, tag="ssq")
                nc.scalar.activation(out=onorm[:], in_=xrow[:], func=Square,
                                     accum_out=ssq[:])
                std = npool.tile([128, 1], F32, tag="std")
                nc.scalar.activation(out=std[:], in_=ssq[:], func=Sqrt,
                                     scale=1.0 / D, bias=epst[:, :1])
                rstd = npool.tile([128, 1], F32, tag="rstd")
                nc.vector.reciprocal(out=rstd[:], in_=std[:])
                nc.vector.tensor_scalar_mul(out=xrow[:], in0=xrow[:],
                                            scalar1=rstd[:, :1])
                nc.vector.tensor_mul(onorm[:], xrow[:], normw_b[:])
                nc.sync.dma_start(out=out_ext[ts(j, 128), :], in_=onorm[:])

    return nc


def _prep_inputs(toks, embed, Wq, Wk, Wv, Wo, norm_w):
    """Build the 8 per-core input maps from the full model inputs."""
    toks = np.asarray(toks)
    embed = np.asarray(embed, dtype=np.float32)
    Wq = np.asarray(Wq, dtype=np.float32)
    Wk = np.asarray(Wk, dtype=np.float32)
    Wv = np.asarray(Wv, dtype=np.float32)
    Wo = np.asarray(Wo, dtype=np.float32)
    norm_w = np.asarray(norm_w, dtype=np.float32)

    inv = 1.0 / (THETA ** (np.arange(0, HD, 2, dtype=np.float32) / HD))  # [32]
    ang = inv[:, None] * np.arange(T, dtype=np.float32)[None, :]         # [32, T]
    cos = np.cos(ang)
    sin = np.sin(ang)
    scale = 1.0 / math.sqrt(HD)
    cosq = np.tile(cos * scale, (4, 1)).astype(np.float32)
    sinq = np.tile(sin * scale, (4, 1)).astype(np.float32)
    cosk = np.tile(cos, (4, 1)).astype(np.float32)
    sink = np.tile(sin, (4, 1)).astype(np.float32)

    jj = np.arange(128)
    trimask = np.where(jj[:, None] <= jj[None, :], 0.0, -1e9).astype(np.float32)

    in_maps = []
    batch_tables = []
    for b in range(B):
        uniq, invmap = np.unique(np.asarray(toks[b], dtype=np.int64),
                                 return_inverse=True)
        table = np.zeros((T, D), dtype=np.float32)
        table[: len(uniq)] = embed[uniq]
        batch_tables.append((table, invmap.astype(np.int32)))

    for c in range(N_CORES):
        b, g = c // 4, c % 4
        table, invmap = batch_tables[b]
        heads = [4 * g + h for h in range(4)]
        ecols = np.concatenate([64 * ah + np.arange(0, 64, 2) for ah in heads])
        ocols = np.concatenate([64 * ah + np.arange(1, 64, 2) for ah in heads])
        vcols = np.arange(256 * g, 256 * g + 256)

        def tile_w(w):  # [L, D, 128 or 256] -> [L, 128, 8, n]
            n = w.shape[-1]
            return np.ascontiguousarray(
                w.reshape(L, 8, 128, n).transpose(0, 2, 1, 3)
            ).astype(ml_dtypes.bfloat16)

        in_maps.append({
            "toks": invmap.reshape(8, 128, 1),
            "emb": table,
            "wqe": tile_w(Wq[:, :, ecols]),
            "wqo": tile_w(Wq[:, :, ocols]),
            "wke": tile_w(Wk[:, :, ecols]),
            "wko": tile_w(Wk[:, :, ocols]),
            "wv": tile_w(Wv[:, :, vcols]),
            "wo": np.ascontiguousarray(
                Wo[:, vcols, :].reshape(L, 4, 64, D)).astype(ml_dtypes.bfloat16),
            "cosq": cosq, "sinq": sinq, "cosk": cosk, "sink": sink,
            "trimask": trimask,
            "normw": norm_w,
        })
    return in_maps


class _Runner:
    """Compile the SPMD program once; re-executable on the 8 cores.

    Mirrors concourse.bass2jax.run_bass_via_pjrt but caches the jitted
    shard_map so repeated calls don't re-trace or re-compile.
    """

    def __init__(self):
        import jax
        from jax.sharding import Mesh, PartitionSpec

        try:
            from jax.experimental.shard_map import shard_map
        except ImportError:
            from jax.shard_map import shard_map

        from concourse import bass2jax

        bass2jax.install_neuronx_cc_hook()
        nc = _build_program()
        self._jax = jax

        partition_name = (
            nc.partition_id_tensor.name if nc.partition_id_tensor else None
        )
        in_names, out_names, out_avals, zero_outs = [], [], [], []
        for alloc in nc.m.functions[0].allocations:
            if not isinstance(alloc, mybir.MemoryLocationSet):
                continue
            name = alloc.memorylocations[0].name
            if alloc.kind == "ExternalInput":
                if name != partition_name:
                    in_names.append(name)
            elif alloc.kind == "ExternalOutput":
                out_names.append(name)
                shape = tuple(alloc.tensor_shape)
                dtype = mybir.dt.np(alloc.dtype)
                out_avals.append(jax.core.ShapedArray(shape, dtype))
                zero_outs.append(np.zeros(shape, dtype))
        self.in_names = list(in_names)
        self.out_names = out_names
        n_params = len(in_names)
        all_in_names = in_names + out_names
        if partition_name is not None:
            all_in_names = all_in_names + [partition_name]

        def _body(*args):
            operands = list(args)
            if partition_name is not None:
                operands.append(bass2jax.partition_id_tensor())
            outs = bass2jax._bass_exec_p.bind(
                *operands,
                out_avals=tuple(out_avals),
                in_names=tuple(all_in_names),
                out_names=tuple(out_names),
                lowering_input_output_aliases=(),
                sim_require_finite=True,
                sim_require_nnan=True,
                nc=nc,
            )
            return tuple(outs)

        devices = jax.devices()[:N_CORES]
        mesh = Mesh(np.asarray(devices), ("core",))
        in_specs = (PartitionSpec("core"),) * (n_params + len(out_names))
        out_specs = (PartitionSpec("core"),) * len(out_names)
        self._fn = jax.jit(
            shard_map(_body, mesh=mesh, in_specs=in_specs,
                      out_specs=out_specs, check_rep=False),
            keep_unused=True,
        )
        self._zero_outs = zero_outs
        self._out_avals = out_avals
        self._mesh = mesh
        self._pspec = PartitionSpec("core")

    def place(self, in_maps, on_device=False):
        cat = [
            np.concatenate([np.asarray(in_maps[c][n]) for c in range(N_CORES)],
                           axis=0)
            for n in self.in_names
        ]
        cat += [
            np.zeros((N_CORES * z.shape[0], *z.shape[1:]), z.dtype)
            for z in self._zero_outs
        ]
        if on_device:
            from jax.sharding import NamedSharding

            sh = NamedSharding(self._mesh, self._pspec)
            cat = [self._jax.device_put(a, sh) for a in cat]
        return cat

    def execute(self, placed):
        return self._fn(*placed)

    def run(self, in_maps):
        out_arrs = self.execute(self.place(in_maps))
        return [
            {
                n: np.asarray(out_arrs[i]).reshape(
                    N_CORES, *self._out_avals[i].shape)[c]
                for i, n in enumerate(self.out_names)
            }
            for c in range(N_CORES)
        ]


_CACHE = {}


def get_runner():
    if "runner" not in _CACHE:
        _CACHE["runner"] = _Runner()
    return _CACHE["runner"]


def kernel(toks, embed, Wq, Wk, Wv, Wo, norm_w):
    in_maps = _prep_inputs(toks, embed, Wq, Wk, Wv, Wo, norm_w)
    results = get_runner().run(in_maps)
    out = np.stack([results[0]["out"], results[4]["out"]])
    return out.astype(np.float32)
